# revision 1
# baseline (speedup 1.0000x reference)
"""Multi-head causal attention block on 8 Trainium2 NeuronCores.

Problem: B=4, S=2048, E=1024, H=16, D=64, causal mask, f32.
Sharding: batch (4) x head-group (2 groups of 8 heads) -> 8 cores.
Core c handles batch b=c//2, heads [8g, 8g+8) with g=c%2.
Megatron layout: qkv col-parallel, out_proj row-parallel; the row-parallel
all-reduce (sum of the 2 head-group partial outputs per batch) is done on
host during the gather, as is the output bias.

Per-core dataflow (bf16 matmul operands, f32 PSUM accumulation):
  xT [E,S] (host pre-transposed, bf16) -+-> QT,KT [512,S] (features on parts)
                                        +-> V natural [S,512] + ones column
  scoresT[k,q] = KT_h.T @ QT_h  (2 heads row-packed in the PE array, K=64 each)
  exp via ACT (scale=1/sqrt(D), bf16 out); no max-subtraction needed
  (|scores| < ~4 for these input scales); causal: fully-masked blocks skipped,
  diagonal blocks masked by DVE multiply with 4 precomputed mask tiles
  attn@V: out[65,q] = V_aug.T @ exp_scoresT  (ones col -> row 64 = denominator)
  normalize: recip(denom) -> PE outer-product broadcast -> DVE multiply
  out proj: y[s,e] = outT.T @ Wout_rows  (contraction over 512 local features)

Emission order pipelines engine-heavy stages: V tiles first, then per
head-pair [QK projection (PE) -> attention (ACT exp-bound)], with the output
projection interleaved after the last pair's attention per q-range, so the
scheduler overlaps PE-heavy projection with ACT-heavy softmax throughout.
"""

import numpy as np

B, S, E, H, D = 4, 2048, 1024, 16, 64
HG = H // 2          # heads per group (8)
NP = HG // 2         # head pairs per group (4)
N_CORES = 8
QT_N = 512           # q tile (free dim) in attention
KT_P = 128           # k tile (partitions) in attention
N_QT = S // QT_N     # 4
N_KT = S // KT_P     # 16
F = HG * D           # local features per core (512)

_CACHE = {}


def _build(n_et, repeat=1, phases="abc"):
    phases, _, flags = phases.partition("!")
    """Build the Bass module. n_et = number of 128-row contraction tiles of
    the (possibly bias-augmented) embedding dim."""
    import concourse.mybir as mybir
    import concourse.tile as tile
    from concourse import bacc

    dt = mybir.dt
    f32, f32r, bf16 = dt.float32, dt.float32r, dt.bfloat16
    AF = mybir.ActivationFunctionType
    E_pad = n_et * 128

    nc = bacc.Bacc("TRN2", target_bir_lowering=False, debug=False,
                   enable_asserts=False, num_devices=N_CORES)

    XT = nc.dram_tensor("xt", [E_pad, S], bf16, kind="ExternalInput").ap()
    WQKV = nc.dram_tensor("wqkv", [E_pad, 3 * F], bf16, kind="ExternalInput").ap()
    WOUT = nc.dram_tensor("wout", [F, E], bf16, kind="ExternalInput").ap()
    DMASK = nc.dram_tensor("dmask", [128, 4, QT_N], bf16, kind="ExternalInput").ap()
    ONES = nc.dram_tensor("ones64", [1, 64], f32r, kind="ExternalInput").ap()
    IDENT = nc.dram_tensor("ident", [128, 128], bf16, kind="ExternalInput").ap()
    Y = nc.dram_tensor("y", [S, E], f32, kind="ExternalOutput").ap()

    with tile.TileContext(nc) as tc, \
         nc.allow_low_precision(reason="bf16 matmul operands by design"):
      for _rep in range(repeat):
        _pipe = "pipe" in flags
        with tc.tile_pool(name="persist", bufs=1) as persist, \
             tc.tile_pool(name="mm_ps", bufs=5 if "narrowexp" in flags else 2,
                          space="PSUM") as mm_ps, \
             tc.tile_pool(name="sp_ps", bufs=2, space="PSUM") as sp_ps, \
             tc.tile_pool(name="at_ps", bufs=1, space="PSUM") as at_ps, \
             tc.tile_pool(name="bc_ps", bufs=1, space="PSUM") as bc_ps, \
             tc.tile_pool(name="exp_sb", bufs=6) as exp_sb, \
             tc.tile_pool(name="nrm_sb", bufs=4) as nrm_sb, \
             tc.tile_pool(name="y_sb", bufs=4) as y_sbp:
            # persistent SBUF tensors (all bf16 except the f32r ones row)
            xk = [persist.tile([128, S], bf16, tag=f"xk{e}", name=f"xk{e}")
                  for e in range(n_et)]
            wqk = [persist.tile([128, 2 * F], bf16, tag=f"wqk{e}", name=f"wqk{e}")
                   for e in range(n_et)]
            wv = [persist.tile([128, F], bf16, tag=f"wv{e}", name=f"wv{e}")
                  for e in range(n_et)]
            wout_sb = [persist.tile([128, E], bf16, tag=f"wo{p}", name=f"wo{p}")
                       for p in range(NP)]
            qt_sb = [persist.tile([128, S], bf16, tag=f"qt{p}", name=f"qt{p}")
                     for p in range(NP)]
            kt_sb = [persist.tile([128, S], bf16, tag=f"kt{p}", name=f"kt{p}")
                     for p in range(NP)]
            vav = [persist.tile([128, HG * (D + 1)], bf16, tag=f"va{k}",
                                name=f"va{k}") for k in range(N_KT)]
            outt = [persist.tile([128, S], bf16, tag=f"ot{p}", name=f"ot{p}")
                    for p in range(NP)]
            masks = persist.tile([128, 4, QT_N], bf16, tag="masks")
            ones_sb = persist.tile([1, 64], f32r, tag="ones")
            ident = persist.tile([128, 128], bf16, tag="ident")

            for e in range(n_et):
                nc.sync.dma_start(xk[e][:], XT[128 * e:128 * (e + 1), :])
            for e in range(n_et):
                nc.sync.dma_start(wqk[e][:], WQKV[128 * e:128 * (e + 1), 0:2 * F])
                nc.sync.dma_start(wv[e][:], WQKV[128 * e:128 * (e + 1), 2 * F:3 * F])
            for p in range(NP):
                nc.sync.dma_start(wout_sb[p][:], WOUT[128 * p:128 * (p + 1), :])
            nc.sync.dma_start(masks[:], DMASK[:])
            nc.sync.dma_start(ones_sb[:], ONES[:])
            nc.sync.dma_start(ident[:], IDENT[:])

            # ---- V tiles (natural layout + ones column), all 16 s-tiles ----
            for st in range(N_KT):
                ps = mm_ps.tile([128, 512], f32, tag="mm", name="mmps")
                for e in range(n_et):
                    nc.tensor.matmul(
                        ps[:],
                        xk[e][:, 128 * st:128 * (st + 1)],
                        wv[e][:],
                        start=(e == 0), stop=(e == n_et - 1))
                va3 = vav[st].rearrange("p (h c) -> p h c", c=D + 1)
                nc.scalar.activation(
                    va3[:, :, 0:D],
                    ps[:].rearrange("p (h c) -> p h c", c=D),
                    AF.Copy)
                nc.any.memset(va3[:, :, D:D + 1], 1.0)

            def emit_qk(p):
                """QT/KT projection for head pair p (features on partitions)."""
                for dest, ft in ((qt_sb[p], p), (kt_sb[p], NP + p)):
                    for sc in range(N_QT):
                        ps = mm_ps.tile([128, 512], f32, tag="mm", name="mmps")
                        for e in range(n_et):
                            nc.tensor.matmul(
                                ps[:],
                                wqk[e][:, 128 * ft:128 * (ft + 1)],
                                xk[e][:, 512 * sc:512 * (sc + 1)],
                                start=(e == 0), stop=(e == n_et - 1))
                        nc.vector.tensor_copy(dest[:, 512 * sc:512 * (sc + 1)],
                                              ps[:])

            def emit_attn(p, qt):
                """Attention for head pair p, queries [512*qt, 512*(qt+1)).

                Two k-tiles of one head share a 2-bank scores PSUM tile
                (each matmul stays inside its own bank) so a single exp
                instruction covers 1024 columns: half the ACT instruction
                overhead and half the ACT->PE handoffs per unit of work."""
                kt_max = (qt + 1) * (QT_N // KT_P)
                apA = at_ps.tile([128, QT_N], f32, tag="apA")
                apB = at_ps.tile([128, QT_N], f32, tag="apB")
                if "narrowexp" in flags:
                    for kt in range(kt_max):
                        for hh, ap in ((0, apA), (1, apB)):
                            lo, hi = 64 * hh, 64 * hh + 64
                            ep = exp_sb.tile([128, QT_N], bf16, tag=f"e{hh}",
                                             name=f"e{hh}")
                            sp = mm_ps.tile([128, QT_N], f32, tag="mm",
                                            name="mmps")
                            dlt = kt - (qt * QT_N) // KT_P
                            diag = dlt >= 0 and "nomask" not in flags
                            nc.tensor.matmul(
                                sp[:],
                                kt_sb[p][lo:hi, 128 * kt:128 * (kt + 1)],
                                qt_sb[p][lo:hi, QT_N * qt:QT_N * (qt + 1)],
                                start=True, stop=not diag)
                            if diag:
                                nc.tensor.matmul(sp[:], ident[:],
                                                 masks[:, dlt, :],
                                                 start=False, stop=True)
                            nc.scalar.activation(
                                ep[:], sp[:],
                                AF.Copy if "noexp" in flags else AF.Exp,
                                scale=float(1.0 / np.sqrt(D)))
                            h = 2 * p + hh
                            nc.tensor.matmul(
                                ap[0:D + 1, :],
                                vav[kt][:, (D + 1) * h:(D + 1) * (h + 1)],
                                ep[:],
                                start=(kt == 0), stop=(kt == kt_max - 1))
                else:
                  for kt2 in range(kt_max // 2):
                      # Emit both heads' score matmuls first, then both exps,
                      # then both attn@V pairs: head A's exp latency hides
                      # under head B's scores on the in-order PE stream.
                      sps, eps2 = {}, {}
                      for hh in range(2):
                          lo, hi = 64 * hh, 64 * hh + 64
                          sp = sp_ps.tile([128, 2 * QT_N], f32, tag="sp2",
                                          name="sp2")
                          sps[hh] = sp
                          for sub in range(2):
                              kt = 2 * kt2 + sub
                              half = sp[:, QT_N * sub:QT_N * (sub + 1)]
                              dlt = kt - (qt * QT_N) // KT_P
                              diag = dlt >= 0 and "nomask" not in flags
                              nc.tensor.matmul(
                                  half,
                                  kt_sb[p][lo:hi, 128 * kt:128 * (kt + 1)],
                                  qt_sb[p][lo:hi, QT_N * qt:QT_N * (qt + 1)],
                                  start=True, stop=not diag)
                              if diag:
                                  # additive causal mask (0 / -240) folded into
                                  # the PSUM group: out += I.T @ maskbias
                                  nc.tensor.matmul(half, ident[:],
                                                   masks[:, dlt, :],
                                                   start=False, stop=True)
                      for hh in range(2):
                          ep = exp_sb.tile([128, 2 * QT_N], bf16, tag=f"e{hh}",
                                           name=f"e{hh}")
                          eps2[hh] = ep
                          nc.scalar.activation(
                              ep[:], sps[hh][:],
                              AF.Copy if "noexp" in flags else AF.Exp,
                              scale=float(1.0 / np.sqrt(D)))
                      for hh, ap in ((0, apA), (1, apB)):
                          h = 2 * p + hh
                          for sub in range(2):
                              kt = 2 * kt2 + sub
                              nc.tensor.matmul(
                                  ap[0:D + 1, :],
                                  vav[kt][:, (D + 1) * h:(D + 1) * (h + 1)],
                                  eps2[hh][:, QT_N * sub:QT_N * (sub + 1)],
                                  start=(kt == 0), stop=(kt == kt_max - 1))
                for hh, ap in ((0, apA), (1, apB)):
                    # (an off-path variant evacuating the accumulator through
                    # an extra ACT copy measured ~90us slower head-to-head --
                    # ACT is the busy engine in this phase; keep the direct
                    # PSUM-consuming chain.)
                    rec = nrm_sb.tile([1, QT_N], f32r, tag="rec")
                    nc.vector.reciprocal(rec[:], ap[D:D + 1, :])
                    bps = (bc_ps.tile([64, QT_N], f32, tag="bps", name="bps")
                           if "narrowexp" in flags else
                           mm_ps.tile([64, QT_N], f32, tag="mm", name="mmps"))
                    nc.tensor.matmul(bps[:], ones_sb[:], rec[:],
                                     start=True, stop=True)
                    bsb = nrm_sb.tile([64, QT_N], f32, tag="bsb")
                    nc.vector.tensor_copy(bsb[:], bps[:])
                    nc.vector.tensor_mul(
                        outt[p][64 * hh:64 * hh + 64, QT_N * qt:QT_N * (qt + 1)],
                        ap[0:D, :], bsb[:])

            def emit_proj(qt):
                """Output projection for s-tiles in q-range qt (all pairs)."""
                for st in range(4 * qt, 4 * (qt + 1)):
                    for et in range(E // 512):
                        ps = mm_ps.tile([128, 512], f32, tag="mm", name="mmps")
                        for p in range(NP):
                            nc.tensor.matmul(
                                ps[:],
                                outt[p][:, 128 * st:128 * (st + 1)],
                                wout_sb[p][:, 512 * et:512 * (et + 1)],
                                start=(p == 0), stop=(p == NP - 1))
                        ysb = y_sbp.tile([128, 512], f32, tag="ysb")
                        nc.vector.tensor_copy(ysb[:], ps[:])
                        nc.sync.dma_start(
                            Y[128 * st:128 * (st + 1), 512 * et:512 * (et + 1)],
                            ysb[:])

            if phases == "a":
                for p in range(NP):
                    emit_qk(p)
                for p in range(NP):
                    for half in range(2):
                        nc.sync.dma_start(
                            Y[(2 * p + half) * 128:(2 * p + half + 1) * 128, :]
                            .bitcast(bf16),
                            (qt_sb[p] if half == 0 else kt_sb[p])[:, :])
                continue

            for p in range(NP):
                emit_qk(p)
                for qt in range(N_QT):
                    emit_attn(p, qt)
                    if p == NP - 1 and phases == "abc":
                        emit_proj(qt)

            if phases == "ab":
                for p in range(NP):
                    for half in range(2):
                        nc.sync.dma_start(
                            Y[(2 * p + half) * 128:(2 * p + half + 1) * 128, :]
                            .bitcast(bf16),
                            outt[p][:, :])

    nc.compile()
    return nc


def _get_nc(n_et, repeat=1, phases="abc"):
    key = (n_et, repeat, phases)
    if key not in _CACHE:
        _CACHE[key] = _build(n_et, repeat, phases)
    return _CACHE[key]


def _shard(x, mask, Wqkv, bqkv, Wout, bout):
    """Host-side sharding: per-core input dicts."""
    import ml_dtypes

    bf16 = ml_dtypes.bfloat16
    x = np.asarray(x, dtype=np.float32)
    mask = np.asarray(mask)
    Wqkv = np.asarray(Wqkv, dtype=np.float32)
    bqkv = np.asarray(bqkv, dtype=np.float32)
    Wout = np.asarray(Wout, dtype=np.float32)

    has_bias = bool(np.any(bqkv))
    n_et = 9 if has_bias else 8
    E_pad = n_et * 128

    # diagonal mask tiles (additive): 0 where allowed, -240 where masked,
    # so exp(scale*(s - 240)) ~ 1e-13 kills masked contributions.
    # dmask[i, d, j] corresponds to mask[j, 128*d + i].
    dmask = np.stack([np.asarray(mask[0:QT_N, 128 * d:128 * (d + 1)].T)
                      for d in range(4)], axis=1).astype(np.float32)
    dmask = ((dmask - 1.0) * 240.0).astype(bf16)
    dmask = np.ascontiguousarray(dmask)  # [128, 4, 512]

    in_maps = []
    for c in range(N_CORES):
        b, g = divmod(c, 2)
        heads = range(HG * g, HG * (g + 1))
        # per-group weight slices, feature order [Q heads | K heads | V heads]
        cols = []
        for blk in range(3):  # q, k, v blocks of Wqkv
            for h in heads:
                cols.append(Wqkv[:, blk * E + D * h: blk * E + D * h + D])
        wqkv_c = np.concatenate(cols, axis=1)  # [E, 3F]
        if has_bias:
            bias_cols = []
            for blk in range(3):
                for h in heads:
                    bias_cols.append(bqkv[blk * E + D * h: blk * E + D * h + D])
            brow = np.concatenate(bias_cols)[None, :]  # [1, 3F]
            wqkv_c = np.concatenate(
                [wqkv_c, brow, np.zeros((E_pad - E - 1, 3 * F), np.float32)], axis=0)
        xt_c = np.ascontiguousarray(x[b].T)  # [E, S]
        if has_bias:
            aug = np.zeros((E_pad - E, S), np.float32)
            aug[0, :] = 1.0
            xt_c = np.concatenate([xt_c, aug], axis=0)
        wout_c = np.ascontiguousarray(Wout[F * g:F * (g + 1), :])  # [F, E]
        in_maps.append({
            "xt": np.ascontiguousarray(xt_c.astype(bf16)),
            "wqkv": np.ascontiguousarray(wqkv_c.astype(bf16)),
            "wout": np.ascontiguousarray(wout_c.astype(bf16)),
            "dmask": dmask,
            "ones64": np.ones((1, 64), np.float32),
            "ident": np.eye(128, dtype=bf16),
        })
    return in_maps, n_et


def run_sharded(inputs, trace=False):
    """Run the SPMD kernel; returns (y_full [B,S,E] f32, BassKernelResults)."""
    from concourse.bass_utils import run_bass_kernel_spmd

    in_maps, n_et = _shard(**inputs)
    nc = _get_nc(n_et)
    res = run_bass_kernel_spmd(nc, in_maps, core_ids=list(range(N_CORES)),
                               trace=trace)
    bout = np.asarray(inputs["bout"], dtype=np.float32)
    y = np.empty((B, S, E), np.float32)
    for b in range(B):
        y[b] = (res.results[2 * b]["y"] + res.results[2 * b + 1]["y"] + bout)
    return y, res


def kernel(**inputs) -> np.ndarray:
    y, _ = run_sharded(inputs, trace=False)
    return y

